# revision 1
# baseline (speedup 1.0000x reference)
"""Trainium2 Bass kernel for nn_KB_Mapping_19361712570541 (dense_cnn).

Math (from the reference, with the W=1 image dimension folded away):
  x: [N=131072, C=128]; work in channels-on-partition layout h = x.T [C, N].
  dw3(h, w)[c,n] = w[c,0]*h[c,n-1] + w[c,1]*h[c,n] + w[c,2]*h[c,n+1]   (zero pad)
  b1 = relu(W1pw @ relu(dw3(h, wd1)))
  b2 = (relu(W21x1 @ h) + b1) * mask
  b2 = relu(W2pw @ relu(dw3(b2, wd2)))
  out = relu(Wf[:, :C] @ h + Wf[:, C:] @ b2)          -> out.T is [N, C]

Sharding: data-parallel along N across 8 cores; each core's input slab
carries a 2-column halo of x and a 1-column halo of the mask, so no
cross-core communication is needed (halo intermediates are recomputed).
Mask is zero-padded at the global edges, which exactly reproduces the
reference's zero padding of the second depthwise conv's input.

On-chip: fp16 operands end-to-end (fp32 PSUM accumulation; measured
end-to-end rel err ~5e-4). Pointwise convs are TensorE matmuls (weights
stationary as [I, O]); depthwise 3-taps are diag-matrix matmuls
accumulated in PSUM. The six PSUM->SBUF elementwise materializations
are split DVE {d1 relu, relu+add (STT), d2 relu} / ACT {b1 relu, b2
relu, out relu}; the mask multiply runs on GPSIMD (SBUF-only engine).
Per-tile input/output DMAs (G=1 beat wider chunks once PSUM was tuned).
PSUM: the dw1 accumulator and the fusion accumulator are double-buffered
(2+2+1+1+1+1 = 8 banks); SBUF working tiles 24-deep. Engine busy is
balanced (PE ~95us, ACT ~90us, DVE ~90us per core in the cost-model
timeline; ~101us end-to-end estimate per core). A single narrow
(256-col) leading tile fills the pipeline faster than a uniform grid;
the relu+add and mask-multiply are split into 4B-aligned 1/4-3/4
pieces so GPSIMD's first-piece multiply overlaps VectorE's larger
second-piece relu+add (uneven because a GPSIMD piece runs ~1.4x a
VectorE piece; 50/50 and 3-piece splits are both slower). Exec sits
within ~1% of VectorE's busy time -- the five PSUM->SBUF relus plus
the relu+add, which only ScalarE/VectorE can run, are the balanced
floor of this design (~100us end-to-end per core).
"""

import numpy as np
from contextlib import ExitStack

import concourse.bass as bass
import concourse.bacc as bacc
import concourse.tile as tile
import concourse.mybir as mybir
from concourse.bass_utils import run_bass_kernel_spmd

C = 128
N = 131072
NCORES = 8
NSH = N // NCORES          # 16384 output columns per core
T = 510                    # full-tile output width
E = T + 2                  # halo-1 intermediate width (= 512, one PSUM bank)
WH = T + 4                 # h tile width
MASK_SEED = 42
MASK_P = 0.5

F32 = mybir.dt.float32
F16 = mybir.dt.float16

LAST_RESULT = None         # BassKernelResults of the most recent run (for test.py)
TRACE = False

_mask_cache = None


def _mask_cn() -> np.ndarray:
    """The reference's fixed Bernoulli mask in [C, N] layout, float16."""
    global _mask_cache
    if _mask_cache is None:
        import jax
        cpu = jax.devices("cpu")[0]
        with jax.default_device(cpu):
            m = jax.random.bernoulli(
                jax.random.key(MASK_SEED), 1.0 - MASK_P, (1, C, N, 1)
            )
            m = np.asarray(m)[0, :, :, 0]
        _mask_cache = m.astype(np.float16)
    return _mask_cache


def _build_nc():
    nc = bacc.Bacc("TRN2", target_bir_lowering=False)

    x_t = nc.dram_tensor("x_t", [C, NSH + 4], F16, kind="ExternalInput")
    mk = nc.dram_tensor("mk", [C, NSH + 2], F16, kind="ExternalInput")
    # 11 stacked [128, 128] weight blocks, each already in lhsT ([K, M]) layout:
    # 0..2 diag(w_b1_dw taps), 3..5 diag(w_b2_dw taps),
    # 6 W1pw^T, 7 W21x1^T, 8 W2pw^T, 9 Wf[:, :C]^T, 10 Wf[:, C:]^T
    w_all = nc.dram_tensor("w_all", [11 * C, C], F16, kind="ExternalInput")
    # dw tap scalars, one per partition: row k of [6, C] = tap k (dw1 0..2, dw2 3..5)
    tp = nc.dram_tensor("tp", [6, C], F32, kind="ExternalInput")
    y_t = nc.dram_tensor("y_t", [C, NSH], F16, kind="ExternalOutput")

    D1_0, D1_1, D1_2, D2_0, D2_1, D2_2, W1PW, W21, W2PW, WFH, WFB = range(11)

    with ExitStack() as ctx:
        tc = ctx.enter_context(tile.TileContext(nc))
        wpool = ctx.enter_context(tc.tile_pool(name="weights", bufs=1))
        sb = ctx.enter_context(tc.tile_pool(name="sbuf", bufs=24))
        sbc = ctx.enter_context(tc.tile_pool(name="sbufc", bufs=10))
        ps_dw = ctx.enter_context(tc.tile_pool(name="ps_dw", bufs=2, space="PSUM"))
        ps_mm = ctx.enter_context(tc.tile_pool(name="ps_mm", bufs=1, space="PSUM"))

        w_sb = wpool.tile([C, 11 * C], F16)
        for k in range(11):
            nc.sync.dma_start(
                out=w_sb[:, k * C:(k + 1) * C], in_=w_all[k * C:(k + 1) * C, :]
            )
        tp_sb = wpool.tile([C, 6], F32)
        nc.sync.dma_start(out=tp_sb[:, :], in_=tp.rearrange("k c -> c k"))

        def w(k):
            return w_sb[:, k * C:(k + 1) * C]

        # graduated tile widths: narrow leading tiles fill the pipeline
        # sooner; steady state runs at the full 510 (PSUM-bank-limited) width
        widths = [256]
        rest = NSH - sum(widths)
        widths += [T] * (rest // T)
        if rest % T:
            widths.append(rest % T)
        assert sum(widths) == NSH
        a = 0
        for i, wT in enumerate(widths):
            wE = wT + 2
            wh = wT + 4
            la = 0

            h_c = sbc.tile([C, T + 4], F16, tag="hc")
            nc.sync.dma_start(out=h_c[:, :wh], in_=x_t[:, a:a + wh])
            mk_c = sbc.tile([C, T + 2], F16, tag="mkc")
            nc.sync.dma_start(out=mk_c[:, :wE], in_=mk[:, a:a + wE])
            o_c = sbc.tile([C, T], F16, tag="oc")
            h_t = h_c
            mk_t = mk_c

            # branch 1: depthwise taps accumulate in PSUM (PE), relu on DVE
            d1p = ps_dw.tile([C, E], F32, tag="d1", name="d1p")
            for m in range(3):
                nc.tensor.matmul(
                    d1p[:, :wE], w(D1_0 + m), h_t[:, m:m + wE],
                    start=(m == 0), stop=(m == 2),
                )
            d1s = sb.tile([C, E], F16, tag="d1s")
            nc.vector.tensor_scalar_max(d1s[:, :wE], d1p[:, :wE], 0.0)

            b1p = ps_mm.tile([C, E], F32, tag="b1", name="b1p")
            nc.tensor.matmul(b1p[:, :wE], w(W1PW), d1s[:, :wE],
                             start=True, stop=True)
            b1r = sb.tile([C, E], F16, tag="b1r")
            nc.scalar.activation(b1r[:, :wE], b1p[:, :wE],
                                 mybir.ActivationFunctionType.Relu)

            # branch 2 head: pointwise, then fused relu+add (DVE), mask (Pool)
            b2ap = ps_mm.tile([C, E], F32, tag="b2a", name="b2ap")
            nc.tensor.matmul(b2ap[:, :wE], w(W21), h_t[:, 1:1 + wE],
                             start=True, stop=True)
            # split relu+add and mask into halves so GPSIMD's left-half
            # multiply overlaps DVE's right-half relu+add
            b2b = sb.tile([C, E], F16, tag="b2b")
            b2m = sb.tile([C, E], F16, tag="b2m")
            hw_ = (wE + 7) // 8 * 2    # 4B-aligned 1/4-3/4 split: the small
                                       # first piece starts Pool sooner (DVE
                                       # piece ~530ns vs Pool's ~740ns)
            for lo, hi in ((0, hw_), (hw_, wE)):
                nc.vector.scalar_tensor_tensor(
                    b2b[:, lo:hi], b2ap[:, lo:hi], 0.0, b1r[:, lo:hi],
                    mybir.AluOpType.max, mybir.AluOpType.add,
                )
                nc.gpsimd.tensor_mul(b2m[:, lo:hi], b2b[:, lo:hi],
                                     mk_t[:, lo:hi])

            # branch 2 tail: depthwise taps (PE), relu (DVE), pointwise, relu
            d2p = ps_mm.tile([C, E], F32, tag="d2", name="d2p")
            for m in range(3):
                nc.tensor.matmul(
                    d2p[:, :wT], w(D2_0 + m), b2m[:, m:m + wT],
                    start=(m == 0), stop=(m == 2),
                )
            d2s = sb.tile([C, E], F16, tag="d2s")
            nc.vector.tensor_scalar_max(d2s[:, :wT], d2p[:, :wT], 0.0)

            b2p = ps_mm.tile([C, E], F32, tag="b2", name="b2p")
            nc.tensor.matmul(b2p[:, :wT], w(W2PW), d2s[:, :wT],
                             start=True, stop=True)
            b2r = sb.tile([C, E], F16, tag="b2r")
            nc.scalar.activation(b2r[:, :wT], b2p[:, :wT],
                                 mybir.ActivationFunctionType.Relu)

            # fusion: two accumulating matmuls, relu on DVE, store per chunk
            fp = ps_dw.tile([C, E], F32, tag="f", name="fp")
            nc.tensor.matmul(fp[:, :wT], w(WFH), h_t[:, 2:2 + wT],
                             start=True, stop=False)
            nc.tensor.matmul(fp[:, :wT], w(WFB), b2r[:, :wT],
                             start=False, stop=True)
            nc.scalar.activation(o_c[:, la:la + wT], fp[:, :wT],
                                 mybir.ActivationFunctionType.Relu)

            nc.sync.dma_start(out=y_t[:, a:a + wT], in_=o_c[:, :wT])
            a += wT

    nc.compile()
    return nc


_nc_cache = None


def kernel(x, w_b1_dw, w_b1_pw, w_b2_1x1, w_b2_dw, w_b2_pw, w_fusion):
    global LAST_RESULT, _nc_cache

    x = np.asarray(x, dtype=np.float32)
    mask = _mask_cn()

    # host-side shard prep: [C, N] layouts with zero-padded halos, fp16
    xt_pad = np.zeros((C, N + 4), dtype=np.float16)
    xt_pad[:, 2:N + 2] = x.T.astype(np.float16)
    mk_pad = np.zeros((C, N + 2), dtype=np.float16)
    mk_pad[:, 1:N + 1] = mask

    def taps(wdw):  # [C,1,3,3] -> 3 diag matrices in lhsT layout
        return [np.diag(np.asarray(wdw)[:, 0, k, 1]).astype(np.float16).T
                for k in range(3)]

    blocks = (
        taps(w_b1_dw) + taps(w_b2_dw) + [
            np.asarray(w_b1_pw)[:, :, 0, 0].T,
            np.asarray(w_b2_1x1)[:, :, 0, 0].T,
            np.asarray(w_b2_pw)[:, :, 0, 0].T,
            np.asarray(w_fusion)[:, :C, 0, 0].T,
            np.asarray(w_fusion)[:, C:, 0, 0].T,
        ]
    )
    w_all = np.ascontiguousarray(
        np.concatenate([b.astype(np.float16) for b in blocks], axis=0)
    )
    tp_arr = np.ascontiguousarray(np.concatenate([
        np.asarray(w_b1_dw)[:, 0, :, 1].T, np.asarray(w_b2_dw)[:, 0, :, 1].T,
    ], axis=0).astype(np.float32))

    in_maps = []
    for i in range(NCORES):
        s = i * NSH
        in_maps.append({
            "x_t": np.ascontiguousarray(xt_pad[:, s:s + NSH + 4]),
            "mk": np.ascontiguousarray(mk_pad[:, s:s + NSH + 2]),
            "w_all": w_all,
            "tp": tp_arr,
        })

    if _nc_cache is None:
        _nc_cache = _build_nc()

    res = run_bass_kernel_spmd(
        _nc_cache, in_maps, core_ids=list(range(NCORES)), trace=TRACE
    )
    LAST_RESULT = res

    out = np.empty((C, N), dtype=np.float32)
    for i in range(NCORES):
        out[:, i * NSH:(i + 1) * NSH] = res.results[i]["y_t"].astype(np.float32)
    return np.ascontiguousarray(out.T)



# revision 21
# speedup vs baseline: 1.3026x; 1.3026x over previous
"""Trainium2 Bass kernel for nn_KB_Mapping_19361712570541 (dense_cnn).

Math (W=1 image dim folded away; h = x.T in [C, N] channels-on-partition):
  dw3(h, w)[c,n] = w0[c]h[c,n-1] + w1[c]h[c,n] + w2[c]h[c,n+1]   (zero pad)
  b1  = relu(W1pw @ relu(dw3(h, wd1)))
  b2  = (relu(W21x1 @ h) + b1) * mask
  b2  = relu(W2pw @ relu(dw3(b2, wd2)))
  out = relu(Wf[:, :C] @ h + Wf[:, C:] @ b2)          -> out.T is [N, C]

Sharding: data-parallel along N across 8 cores; each core's input slab
carries halos of x/mask so no cross-core communication is needed.

Implementation notes (cost-model driven):
- All matmuls except the fusion's Wfh@h run as fp8e4 DoubleRow pairs
  (0.5 cycles/row, two K=128 planes per instruction). Depthwise taps
  pair {t0,t2} (plane stride +2 — odd plane strides are rejected by
  codegen) with {t1, zero-plane} as the second DR; single GEMMs ride a
  zero second weight plane at half cost. The accuracy-critical fusion
  term Wfh@h runs in fp16 (x shipped both as fp8 and fp16); measured
  end-to-end rel err ~6e-4.
- Six PSUM->SBUF evacuations per 510-col tile are the floor: DVE takes
  {relu d1, relu+add (STT), relu d2}, ACT {relu b1, relu b2, relu out};
  the mask multiply runs on Pool. One PSUM bank per tensor (d1 and the
  fusion accumulator double-buffered; 8 banks total).
- The emission is software-pipelined: iteration k emits the front half
  of tile k (dw1/d1s/b1p/b1r), then the back-dve of k-1 (dw2/d2s), the
  head of k (b2a/STT/mask), the back of k-1 (b2p/b2r/fusion), and the
  output relu of k-2 — so every engine's in-order queue has ready work
  while other engines fill a tile's serial dependency chain.
- DMAs are batched into graduated groups because each dma_start costs
  ~625ns of serial HWDGE time.
- Zero-plane DRs read 2 columns past the producer's content; those
  tiles are 516 wide with a one-time memset of the tail columns.
"""

import numpy as np
from contextlib import ExitStack

import ml_dtypes

import concourse.bass as bass
import concourse.bacc as bacc
import concourse.tile as tile
import concourse.mybir as mybir
from concourse.ap import AP
from concourse.bass_utils import run_bass_kernel_spmd

C = 128
N = 131072
NCORES = 8
NSH = N // NCORES          # 16384 output columns per core
T = 510                    # steady-state tile width (wE = 512 = one PSUM bank)
MASK_SEED = 42
MASK_P = 0.5

F32 = mybir.dt.float32
F16 = mybir.dt.float16
F8 = mybir.dt.float8e4
NP8 = ml_dtypes.float8_e4m3
DR = mybir.MatmulPerfMode.DoubleRow
Relu = mybir.ActivationFunctionType.Relu

# DR weight-pair indices in w8 (each pair is [C, 2, C] -> 256 cols)
P_D1A, P_D1B, P_W1, P_W21, P_D2A, P_D2B, P_W2, P_WFB = range(8)

LAST_RESULT = None         # BassKernelResults of the most recent run (for test.py)
TRACE = False

_mask_cache = None
_nc_cache = None


def _mask_cn() -> np.ndarray:
    """The reference's fixed Bernoulli mask in [C, N] layout, float32."""
    global _mask_cache
    if _mask_cache is None:
        import jax
        cpu = jax.devices("cpu")[0]
        with jax.default_device(cpu):
            m = jax.random.bernoulli(
                jax.random.key(MASK_SEED), 1.0 - MASK_P, (1, C, N, 1)
            )
            _mask_cache = np.asarray(m)[0, :, :, 0].astype(np.float32)
    return _mask_cache


def _tiles():
    """(a, width) list covering [0, NSH); narrow leader fills the pipe and
    narrow trailers drain it."""
    widths = [256] + [T] * 30 + [276] * 3
    assert sum(widths) == NSH
    out, a = [], 0
    for w in widths:
        out.append((a, w))
        a += w
    return out


def _groups(tiles):
    """Graduated DMA groups as slices of the tile list."""
    sizes = [2, 2, 4, 8, 8, 9]
    gs, i = [], 0
    for s in sizes:
        if i >= len(tiles):
            break
        gs.append(tiles[i:i + s])
        i += s
    if i < len(tiles):
        gs.append(tiles[i:])
    return gs


def _dr_rhs(t, col, n, delta=2):
    """[C, 2, n] moving AP over tile t: plane0 at col, plane1 at col+delta."""
    base = t[:, col:col + n]
    return AP(base.tensor, base.offset,
              [list(base.ap[0]), [delta, 2], [1, n]])


def _build_nc():
    nc = bacc.Bacc("TRN2", target_bir_lowering=False)

    x8 = nc.dram_tensor("x8", [C, NSH + 8], F8, kind="ExternalInput")
    x16 = nc.dram_tensor("x16", [C, NSH], F16, kind="ExternalInput")
    mk = nc.dram_tensor("mk", [C, NSH + 2], F8, kind="ExternalInput")
    w8 = nc.dram_tensor("w8", [C, 8 * 2 * C], F8, kind="ExternalInput")
    wf16 = nc.dram_tensor("wf16", [C, C], F16, kind="ExternalInput")
    y = nc.dram_tensor("y", [C, NSH], F16, kind="ExternalOutput")

    tiles = _tiles()
    groups = _groups(tiles)
    gw_max = max(sum(w for _, w in g) for g in groups)
    group_of = {}
    for gi, g in enumerate(groups):
        for t_ in g:
            group_of[t_[0]] = gi

    with ExitStack() as ctx:
        tc = ctx.enter_context(tile.TileContext(nc))
        wpool = ctx.enter_context(tc.tile_pool(name="weights", bufs=1))
        slab = ctx.enter_context(tc.tile_pool(name="slab", bufs=3))
        opool = ctx.enter_context(tc.tile_pool(name="out", bufs=2))
        work = ctx.enter_context(tc.tile_pool(name="work", bufs=3))
        ps = ctx.enter_context(tc.tile_pool(name="ps", bufs=1, space="PSUM"))

        w8_sb = wpool.tile([C, 8 * 2 * C], F8)
        wf_sb = wpool.tile([C, C], F16)

        def wpair(k):
            return w8_sb[:, k * 2 * C:(k + 1) * 2 * C].rearrange(
                "p (two m) -> p two m", two=2)

        # One-time zero of every buffer whose tail columns are read by
        # zero-plane DRs (delta-2 planes reach 2 cols past the written
        # content; the interpreter hard-faults on uninitialized reads).
        ZBUFS = {"d1s": 2, "b2m": 3, "d2s": 2, "b2r": 2}
        for ztag, zb in ZBUFS.items():
            for _ in range(zb):
                zt = work.tile([C, 516], F8, tag=ztag, bufs=zb,
                               name=f"z_{ztag}")
                nc.gpsimd.memset(zt[:, :], 0.0)

        # per-group slab state
        cur = {}

        pending_hs = []

        def load_group(gi):
            g = groups[gi]
            ga = g[0][0]
            gw = sum(w for _, w in g)
            xs = slab.tile([C, gw_max + 6], F8, tag="xs")
            nc.sync.dma_start(out=xs[:, :gw + 6], in_=x8[:, ga:ga + gw + 6])
            if gi == 0:
                # interleave the weight loads so dw1's pairs (first 768
                # cols) land right after the first x slab: the leading
                # tile's matmuls start ~2 HWDGE slots in.
                nc.sync.dma_start(out=w8_sb[:, :768], in_=w8[:, :768])
            ms = slab.tile([C, gw_max + 2], F8, tag="ms")
            nc.sync.dma_start(out=ms[:, :gw + 2], in_=mk[:, ga:ga + gw + 2])
            if gi == 0:
                nc.sync.dma_start(out=w8_sb[:, 768:], in_=w8[:, 768:])
            # the fp16 slab is only read by the fusion (lag 3): defer its
            # DMA so the next group's critical x slab gets the HWDGE first
            hs = slab.tile([C, gw_max], F16, tag="hs")
            o_c = opool.tile([C, gw_max], F16, tag="oc")
            cur[gi] = dict(ga=ga, gw=gw, xs=xs, hs=hs, ms=ms, o_c=o_c,
                           flushed=0, done=0)
            pending_hs.append(gi)

        def flush_hs():
            while pending_hs:
                gi = pending_hs.pop(0)
                g = cur[gi]
                nc.sync.dma_start(out=g["hs"][:, :g["gw"]],
                                  in_=x16[:, g["ga"]:g["ga"] + g["gw"]])
                if gi == 0:
                    nc.sync.dma_start(out=wf_sb[:, :], in_=wf16[:, :])

        def front_a(st):
            """dw1 -> d1s (DVE) -> b1p -> b1r (ACT)."""
            g = cur[st["gi"]]
            la, wE = st["la"], st["wE"]
            xs = g["xs"]
            d1p = ps.tile([C, 512], F32, tag="d1", bufs=2, name="d1p")
            nc.tensor.matmul(d1p[:, :wE], wpair(P_D1A),
                             _dr_rhs(xs, la, wE),
                             start=True, stop=False, perf_mode=DR)
            nc.tensor.matmul(d1p[:, :wE], wpair(P_D1B),
                             _dr_rhs(xs, la + 1, wE),
                             start=False, stop=True, perf_mode=DR)
            d1s = work.tile([C, 516], F8, tag="d1s", bufs=2)
            nc.vector.tensor_scalar_max(d1s[:, :wE], d1p[:, :wE], 0.0)
            b1p = ps.tile([C, 512], F32, tag="b1", name="b1p")
            nc.tensor.matmul(b1p[:, :wE], wpair(P_W1), _dr_rhs(d1s, 0, wE),
                             start=True, stop=True, perf_mode=DR)
            b1r = work.tile([C, 512], F16, tag="b1r", bufs=2)
            nc.scalar.activation(b1r[:, :wE], b1p[:, :wE], Relu)
            st.update(b1r=b1r)

        def back_dve(st):
            """dw2 -> d2s (DVE; ACT on a couple of tiles for balance)."""
            P_ = st["P_"]
            d2p = ps.tile([C, 512], F32, tag="d2", name="d2p")
            b2m = st["b2m"]
            nc.tensor.matmul(d2p[:, :P_], wpair(P_D2A), _dr_rhs(b2m, 0, P_),
                             start=True, stop=False, perf_mode=DR)
            nc.tensor.matmul(d2p[:, :P_], wpair(P_D2B), _dr_rhs(b2m, 1, P_),
                             start=False, stop=True, perf_mode=DR)
            d2s = work.tile([C, 516], F8, tag="d2s", bufs=2)
            if st.get("d2s_on_act"):
                nc.scalar.activation(d2s[:, :P_], d2p[:, :P_], Relu)
            else:
                nc.vector.tensor_scalar_max(d2s[:, :P_], d2p[:, :P_], 0.0)
            st.update(d2s=d2s)

        def front_b(st):
            """b2a -> STT (DVE) -> mask (Pool)."""
            g = cur[st["gi"]]
            la, wE = st["la"], st["wE"]
            b2ap = ps.tile([C, 512], F32, tag="b2a", name="b2ap")
            nc.tensor.matmul(b2ap[:, :wE], wpair(P_W21),
                             _dr_rhs(g["xs"], la + 1, wE),
                             start=True, stop=True, perf_mode=DR)
            b2b = work.tile([C, 512], F16, tag="b2b", bufs=2)
            nc.vector.scalar_tensor_tensor(
                b2b[:, :wE], b2ap[:, :wE], 0.0, st["b1r"][:, :wE],
                mybir.AluOpType.max, mybir.AluOpType.add)
            b2m = work.tile([C, 516], F8, tag="b2m", bufs=3)
            nc.gpsimd.tensor_mul(b2m[:, :wE], b2b[:, :wE],
                                 g["ms"][:, la:la + wE])
            st.update(b2m=b2m)

        def back_rest(st):
            """b2p -> b2r (ACT) -> fusion matmuls."""
            g = cur[st["gi"]]
            la, P_ = st["la"], st["P_"]
            b2p = ps.tile([C, 512], F32, tag="b2", name="b2p")
            nc.tensor.matmul(b2p[:, :P_], wpair(P_W2),
                             _dr_rhs(st["d2s"], 0, P_),
                             start=True, stop=True, perf_mode=DR)
            b2r = work.tile([C, 516], F8, tag="b2r", bufs=2)
            if st.get("tail"):
                nc.vector.tensor_scalar_max(b2r[:, :P_], b2p[:, :P_], 0.0)
            else:
                nc.scalar.activation(b2r[:, :P_], b2p[:, :P_], Relu)
            fp = ps.tile([C, 512], F32, tag="f", bufs=2, name="fp")
            nc.tensor.matmul(fp[:, :P_], wf_sb[:, :], g["hs"][:, la:la + P_],
                             start=True, stop=False)
            nc.tensor.matmul(fp[:, :P_], wpair(P_WFB), _dr_rhs(b2r, 0, P_),
                             start=False, stop=True, perf_mode=DR)
            st.update(fp=fp)

        def out_relu(st):
            """final relu (ACT) + output flush bookkeeping."""
            g = cur[st["gi"]]
            la, P_ = st["la"], st["P_"]
            if st.get("tail"):
                nc.vector.tensor_scalar_max(g["o_c"][:, la:la + P_],
                                            st["fp"][:, :P_], 0.0)
            else:
                nc.scalar.activation(g["o_c"][:, la:la + P_], st["fp"][:, :P_],
                                     Relu)
            g["done"] += 1
            ntiles = len(groups[st["gi"]])
            # flush every 2 finished tiles
            if g["done"] % 2 == 0 or g["done"] == ntiles:
                lo, hi = g["flushed"], la + P_
                nc.sync.dma_start(out=y[:, g["ga"] + lo:g["ga"] + hi],
                                  in_=g["o_c"][:, lo:hi])
                g["flushed"] = hi

        # software-pipelined emission with skew: per iteration i the engine
        # queues get  DVE:[d1s(i), d2s(i-3), STT(i-1)]
        #             ACT:[b1r(i), b2r(i-3), o(i-4)]
        # so no engine waits on a cross-engine round trip: the Pool mask op
        # of tile i-3 has had two full iterations to finish before dw2/d2s.
        flat = [t_ for g in groups for t_ in g]
        n = len(flat)
        sts = []
        loaded = 0

        def ensure_loaded(upto):
            nonlocal loaded
            while loaded <= min(upto, len(groups) - 1):
                load_group(loaded)
                loaded += 1

        ensure_loaded(0)
        for i in range(n + 4):
            if i < n:
                a, P_ = flat[i]
                gi = group_of[a]
                ensure_loaded(gi + 1)
                st = dict(a=a, P_=P_, wE=P_ + 2, gi=gi,
                          la=a - cur[gi]["ga"],
                          d2s_on_act=(i in (11, 22)),
                          tail=(i >= n - 3))
                sts.append(st)
                front_a(st)
                flush_hs()
            if 0 <= i - 3 < n:
                back_dve(sts[i - 3])
            if 0 <= i - 1 < n:
                front_b(sts[i - 1])
            if 0 <= i - 3 < n:
                back_rest(sts[i - 3])
            if 0 <= i - 4 < n:
                out_relu(sts[i - 4])

    nc.compile()
    return nc


def kernel(x, w_b1_dw, w_b1_pw, w_b2_1x1, w_b2_dw, w_b2_pw, w_fusion):
    global LAST_RESULT, _nc_cache

    x = np.asarray(x, dtype=np.float32)
    h = np.ascontiguousarray(x.T)
    mask = _mask_cn()

    # host-side shard prep: [C, N] layouts, zero-padded halos
    x8_pad = np.zeros((C, N + 8), dtype=NP8)
    x8_pad[:, 2:N + 2] = h.astype(NP8)
    x16_pad = h.astype(np.float16)
    mk_pad = np.zeros((C, N + 2), dtype=NP8)
    mk_pad[:, 1:N + 1] = mask.astype(NP8)

    def taps(wdw):  # [C,1,3,3] -> per-channel taps along N
        return np.asarray(wdw)[:, 0, :, 1]  # [C, 3]

    t1 = taps(w_b1_dw)
    t2 = taps(w_b2_dw)

    def diag8(v):
        return np.diag(v.astype(np.float32)).astype(NP8)

    def lhsT8(w):  # [O, I] -> [I, O] fp8
        return np.ascontiguousarray(np.asarray(w, dtype=np.float32).T).astype(NP8)

    zero = np.zeros((C, C), dtype=NP8)
    pairs = [
        (diag8(t1[:, 0]), diag8(t1[:, 2])),
        (diag8(t1[:, 1]), zero),
        (lhsT8(np.asarray(w_b1_pw)[:, :, 0, 0]), zero),
        (lhsT8(np.asarray(w_b2_1x1)[:, :, 0, 0]), zero),
        (diag8(t2[:, 0]), diag8(t2[:, 2])),
        (diag8(t2[:, 1]), zero),
        (lhsT8(np.asarray(w_b2_pw)[:, :, 0, 0]), zero),
        (lhsT8(np.asarray(w_fusion)[:, C:, 0, 0]), zero),
    ]
    w8_host = np.empty((C, 8 * 2 * C), dtype=NP8)
    for k, (p0, p1) in enumerate(pairs):
        w8_host[:, (2 * k) * C:(2 * k + 1) * C] = p0
        w8_host[:, (2 * k + 1) * C:(2 * k + 2) * C] = p1
    wf_host = np.ascontiguousarray(
        np.asarray(w_fusion)[:, :C, 0, 0].astype(np.float32).T
    ).astype(np.float16)

    in_maps = []
    for i in range(NCORES):
        s = i * NSH
        in_maps.append({
            "x8": np.ascontiguousarray(x8_pad[:, s:s + NSH + 8]),
            "x16": np.ascontiguousarray(x16_pad[:, s:s + NSH]),
            "mk": np.ascontiguousarray(mk_pad[:, s:s + NSH + 2]),
            "w8": w8_host,
            "wf16": wf_host,
        })

    if _nc_cache is None:
        _nc_cache = _build_nc()

    res = run_bass_kernel_spmd(
        _nc_cache, in_maps, core_ids=list(range(NCORES)), trace=TRACE
    )
    LAST_RESULT = res

    out = np.empty((C, N), dtype=np.float32)
    for i in range(NCORES):
        out[:, i * NSH:(i + 1) * NSH] = res.results[i]["y"].astype(np.float32)
    return np.ascontiguousarray(out.T)


# revision 23
# speedup vs baseline: 1.3051x; 1.0019x over previous
"""Trainium2 Bass kernel for nn_KB_Mapping_19361712570541 (dense_cnn).

Math (W=1 image dim folded away; h = x.T in [C, N] channels-on-partition):
  dw3(h, w)[c,n] = w0[c]h[c,n-1] + w1[c]h[c,n] + w2[c]h[c,n+1]   (zero pad)
  b1  = relu(W1pw @ relu(dw3(h, wd1)))
  b2  = (relu(W21x1 @ h) + b1) * mask
  b2  = relu(W2pw @ relu(dw3(b2, wd2)))
  out = relu(Wf[:, :C] @ h + Wf[:, C:] @ b2)          -> out.T is [N, C]

Sharding: data-parallel along N across 8 cores; each core's input slab
carries halos of x/mask so no cross-core communication is needed.

Implementation notes (cost-model driven):
- All matmuls except the fusion's Wfh@h run as fp8e4 DoubleRow pairs
  (0.5 cycles/row, two K=128 planes per instruction). Depthwise taps
  pair {t0,t2} (plane stride +2 — odd plane strides are rejected by
  codegen) with {t1, zero-plane} as the second DR; single GEMMs ride a
  zero second weight plane at half cost. The accuracy-critical fusion
  term Wfh@h runs in fp16 (x shipped both as fp8 and fp16); measured
  end-to-end rel err ~6e-4.
- Six PSUM->SBUF evacuations per 510-col tile are the floor: DVE takes
  {relu d1, relu+add (STT), relu d2}, ACT {relu b1, relu b2, relu out};
  the mask multiply runs on Pool. One PSUM bank per tensor (d1 and the
  fusion accumulator double-buffered; 8 banks total).
- The emission is software-pipelined with skew: iteration i emits
  front_a(i) (dw1/d1s/b1p/b1r), back_dve(i-3) (dw2/d2s), front_b(i-1)
  (b2a/STT/mask), back_rest(i-3) (b2p/b2r/fusion), out_relu(i-4) — so
  per iteration the in-order queues see DVE:[d1s(i), d2s(i-3),
  STT(i-1)] and ACT:[b1r(i), b2r(i-3), o(i-4)] with every dependency
  (including the ~1.3us Pool mask latency) already satisfied. Two
  mid-stream tiles run d2s on ACT for balance; the last three tiles run
  b2r/o on DVE because ACT serializes the drain.
- DMAs are batched into graduated groups because each dma_start costs
  ~625ns of serial HWDGE time; the fp16 fusion slab rides one iteration
  later than the critical fp8 slab, and a narrow leading tile plus
  narrow trailing tiles shorten pipeline fill/drain.
- Zero-plane DRs read up to 2 columns past the producer's content; the
  fp8 work tiles are 516 wide and fully memset once per buffer (the
  interpreter hard-faults on uninitialized reads).
"""

import numpy as np
from contextlib import ExitStack

import ml_dtypes

import concourse.bass as bass
import concourse.bacc as bacc
import concourse.tile as tile
import concourse.mybir as mybir
from concourse.ap import AP
from concourse.bass_utils import run_bass_kernel_spmd

C = 128
N = 131072
NCORES = 8
NSH = N // NCORES          # 16384 output columns per core
T = 510                    # steady-state tile width (wE = 512 = one PSUM bank)
MASK_SEED = 42
MASK_P = 0.5

F32 = mybir.dt.float32
F16 = mybir.dt.float16
F8 = mybir.dt.float8e4
NP8 = ml_dtypes.float8_e4m3
DR = mybir.MatmulPerfMode.DoubleRow
Relu = mybir.ActivationFunctionType.Relu

# DR weight-pair indices in w8 (each pair is [C, 2, C] -> 256 cols)
P_D1A, P_D1B, P_W1, P_W21, P_D2A, P_D2B, P_W2, P_WFB = range(8)

LAST_RESULT = None         # BassKernelResults of the most recent run (for test.py)
TRACE = False

_mask_cache = None
_nc_cache = None


def _mask_cn() -> np.ndarray:
    """The reference's fixed Bernoulli mask in [C, N] layout, float32."""
    global _mask_cache
    if _mask_cache is None:
        import jax
        cpu = jax.devices("cpu")[0]
        with jax.default_device(cpu):
            m = jax.random.bernoulli(
                jax.random.key(MASK_SEED), 1.0 - MASK_P, (1, C, N, 1)
            )
            _mask_cache = np.asarray(m)[0, :, :, 0].astype(np.float32)
    return _mask_cache


def _tiles():
    """(a, width) list covering [0, NSH); narrow leader fills the pipe and
    narrow trailers drain it."""
    widths = [256] + [T] * 30 + [276] * 3
    assert sum(widths) == NSH
    out, a = [], 0
    for w in widths:
        out.append((a, w))
        a += w
    return out


def _groups(tiles):
    """Graduated DMA groups as slices of the tile list."""
    sizes = [2, 2, 4, 8, 8, 9]
    gs, i = [], 0
    for s in sizes:
        if i >= len(tiles):
            break
        gs.append(tiles[i:i + s])
        i += s
    if i < len(tiles):
        gs.append(tiles[i:])
    return gs


def _dr_rhs(t, col, n, delta=2):
    """[C, 2, n] moving AP over tile t: plane0 at col, plane1 at col+delta."""
    base = t[:, col:col + n]
    return AP(base.tensor, base.offset,
              [list(base.ap[0]), [delta, 2], [1, n]])


def _build_nc():
    nc = bacc.Bacc("TRN2", target_bir_lowering=False)

    x8 = nc.dram_tensor("x8", [C, NSH + 8], F8, kind="ExternalInput")
    x16 = nc.dram_tensor("x16", [C, NSH], F16, kind="ExternalInput")
    mk = nc.dram_tensor("mk", [C, NSH + 2], F8, kind="ExternalInput")
    w8 = nc.dram_tensor("w8", [C, 8 * 2 * C], F8, kind="ExternalInput")
    wf16 = nc.dram_tensor("wf16", [C, C], F16, kind="ExternalInput")
    y = nc.dram_tensor("y", [C, NSH], F16, kind="ExternalOutput")

    tiles = _tiles()
    groups = _groups(tiles)
    gw_max = max(sum(w for _, w in g) for g in groups)
    group_of = {}
    for gi, g in enumerate(groups):
        for t_ in g:
            group_of[t_[0]] = gi

    with ExitStack() as ctx:
        tc = ctx.enter_context(tile.TileContext(nc))
        wpool = ctx.enter_context(tc.tile_pool(name="weights", bufs=1))
        slab = ctx.enter_context(tc.tile_pool(name="slab", bufs=3))
        opool = ctx.enter_context(tc.tile_pool(name="out", bufs=2))
        work = ctx.enter_context(tc.tile_pool(name="work", bufs=3))
        ps = ctx.enter_context(tc.tile_pool(name="ps", bufs=1, space="PSUM"))

        w8_sb = wpool.tile([C, 8 * 2 * C], F8)
        wf_sb = wpool.tile([C, C], F16)

        def wpair(k):
            return w8_sb[:, k * 2 * C:(k + 1) * 2 * C].rearrange(
                "p (two m) -> p two m", two=2)

        # One-time zero of every buffer whose tail columns are read by
        # zero-plane DRs (delta-2 planes reach 2 cols past the written
        # content; the interpreter hard-faults on uninitialized reads).
        ZBUFS = {"d1s": 2, "b2m": 3, "d2s": 2, "b2r": 2}
        for ztag, zb in ZBUFS.items():
            for _ in range(zb):
                zt = work.tile([C, 516], F8, tag=ztag, bufs=zb,
                               name=f"z_{ztag}")
                nc.gpsimd.memset(zt[:, :], 0.0)

        # per-group slab state
        cur = {}

        pending_hs = []

        def load_group(gi):
            g = groups[gi]
            ga = g[0][0]
            gw = sum(w for _, w in g)
            xs = slab.tile([C, gw_max + 6], F8, tag="xs")
            nc.sync.dma_start(out=xs[:, :gw + 6], in_=x8[:, ga:ga + gw + 6])
            if gi == 0:
                # interleave the weight loads so dw1's pairs (first 768
                # cols) land right after the first x slab: the leading
                # tile's matmuls start ~2 HWDGE slots in.
                nc.sync.dma_start(out=w8_sb[:, :512], in_=w8[:, :512])
            ms = slab.tile([C, gw_max + 2], F8, tag="ms")
            nc.sync.dma_start(out=ms[:, :gw + 2], in_=mk[:, ga:ga + gw + 2])
            if gi == 0:
                nc.sync.dma_start(out=w8_sb[:, 512:], in_=w8[:, 512:])
            # the fp16 slab is only read by the fusion (lag 3): defer its
            # DMA so the next group's critical x slab gets the HWDGE first
            hs = slab.tile([C, gw_max], F16, tag="hs")
            o_c = opool.tile([C, gw_max], F16, tag="oc")
            cur[gi] = dict(ga=ga, gw=gw, xs=xs, hs=hs, ms=ms, o_c=o_c,
                           flushed=0, done=0)
            pending_hs.append(gi)

        def flush_hs():
            while pending_hs:
                gi = pending_hs.pop(0)
                g = cur[gi]
                nc.sync.dma_start(out=g["hs"][:, :g["gw"]],
                                  in_=x16[:, g["ga"]:g["ga"] + g["gw"]])
                if gi == 0:
                    nc.sync.dma_start(out=wf_sb[:, :], in_=wf16[:, :])

        def front_a(st):
            """dw1 -> d1s (DVE) -> b1p -> b1r (ACT)."""
            g = cur[st["gi"]]
            la, wE = st["la"], st["wE"]
            xs = g["xs"]
            d1p = ps.tile([C, 512], F32, tag="d1", bufs=2, name="d1p")
            nc.tensor.matmul(d1p[:, :wE], wpair(P_D1A),
                             _dr_rhs(xs, la, wE),
                             start=True, stop=False, perf_mode=DR)
            nc.tensor.matmul(d1p[:, :wE], wpair(P_D1B),
                             _dr_rhs(xs, la + 1, wE),
                             start=False, stop=True, perf_mode=DR)
            d1s = work.tile([C, 516], F8, tag="d1s", bufs=2)
            nc.vector.tensor_scalar_max(d1s[:, :wE], d1p[:, :wE], 0.0)
            b1p = ps.tile([C, 512], F32, tag="b1", name="b1p")
            nc.tensor.matmul(b1p[:, :wE], wpair(P_W1), _dr_rhs(d1s, 0, wE),
                             start=True, stop=True, perf_mode=DR)
            b1r = work.tile([C, 512], F16, tag="b1r", bufs=2)
            nc.scalar.activation(b1r[:, :wE], b1p[:, :wE], Relu)
            st.update(b1r=b1r)

        def back_dve(st):
            """dw2 -> d2s (DVE; ACT on a couple of tiles for balance)."""
            P_ = st["P_"]
            d2p = ps.tile([C, 512], F32, tag="d2", name="d2p")
            b2m = st["b2m"]
            nc.tensor.matmul(d2p[:, :P_], wpair(P_D2A), _dr_rhs(b2m, 0, P_),
                             start=True, stop=False, perf_mode=DR)
            nc.tensor.matmul(d2p[:, :P_], wpair(P_D2B), _dr_rhs(b2m, 1, P_),
                             start=False, stop=True, perf_mode=DR)
            d2s = work.tile([C, 516], F8, tag="d2s", bufs=2)
            if st.get("d2s_on_act"):
                nc.scalar.activation(d2s[:, :P_], d2p[:, :P_], Relu)
            else:
                nc.vector.tensor_scalar_max(d2s[:, :P_], d2p[:, :P_], 0.0)
            st.update(d2s=d2s)

        def front_b(st):
            """b2a -> STT (DVE) -> mask (Pool)."""
            g = cur[st["gi"]]
            la, wE = st["la"], st["wE"]
            b2ap = ps.tile([C, 512], F32, tag="b2a", name="b2ap")
            nc.tensor.matmul(b2ap[:, :wE], wpair(P_W21),
                             _dr_rhs(g["xs"], la + 1, wE),
                             start=True, stop=True, perf_mode=DR)
            b2b = work.tile([C, 512], F16, tag="b2b", bufs=2)
            nc.vector.scalar_tensor_tensor(
                b2b[:, :wE], b2ap[:, :wE], 0.0, st["b1r"][:, :wE],
                mybir.AluOpType.max, mybir.AluOpType.add)
            b2m = work.tile([C, 516], F8, tag="b2m", bufs=3)
            nc.gpsimd.tensor_mul(b2m[:, :wE], b2b[:, :wE],
                                 g["ms"][:, la:la + wE])
            st.update(b2m=b2m)

        def back_rest(st):
            """b2p -> b2r (ACT) -> fusion matmuls."""
            g = cur[st["gi"]]
            la, P_ = st["la"], st["P_"]
            b2p = ps.tile([C, 512], F32, tag="b2", name="b2p")
            nc.tensor.matmul(b2p[:, :P_], wpair(P_W2),
                             _dr_rhs(st["d2s"], 0, P_),
                             start=True, stop=True, perf_mode=DR)
            b2r = work.tile([C, 516], F8, tag="b2r", bufs=2)
            if st.get("tail"):
                nc.vector.tensor_scalar_max(b2r[:, :P_], b2p[:, :P_], 0.0)
            else:
                nc.scalar.activation(b2r[:, :P_], b2p[:, :P_], Relu)
            fp = ps.tile([C, 512], F32, tag="f", bufs=2, name="fp")
            nc.tensor.matmul(fp[:, :P_], wf_sb[:, :], g["hs"][:, la:la + P_],
                             start=True, stop=False)
            nc.tensor.matmul(fp[:, :P_], wpair(P_WFB), _dr_rhs(b2r, 0, P_),
                             start=False, stop=True, perf_mode=DR)
            st.update(fp=fp)

        def out_relu(st):
            """final relu (ACT) + output flush bookkeeping."""
            g = cur[st["gi"]]
            la, P_ = st["la"], st["P_"]
            if st.get("tail"):
                nc.vector.tensor_scalar_max(g["o_c"][:, la:la + P_],
                                            st["fp"][:, :P_], 0.0)
            else:
                nc.scalar.activation(g["o_c"][:, la:la + P_], st["fp"][:, :P_],
                                     Relu)
            g["done"] += 1
            ntiles = len(groups[st["gi"]])
            # flush every 2 finished tiles
            if g["done"] % 2 == 0 or g["done"] == ntiles:
                lo, hi = g["flushed"], la + P_
                nc.sync.dma_start(out=y[:, g["ga"] + lo:g["ga"] + hi],
                                  in_=g["o_c"][:, lo:hi])
                g["flushed"] = hi

        # software-pipelined emission with skew: per iteration i the engine
        # queues get  DVE:[d1s(i), d2s(i-3), STT(i-1)]
        #             ACT:[b1r(i), b2r(i-3), o(i-4)]
        # so no engine waits on a cross-engine round trip: the Pool mask op
        # of tile i-3 has had two full iterations to finish before dw2/d2s.
        flat = [t_ for g in groups for t_ in g]
        n = len(flat)
        sts = []
        loaded = 0

        def ensure_loaded(upto):
            nonlocal loaded
            while loaded <= min(upto, len(groups) - 1):
                load_group(loaded)
                loaded += 1

        ensure_loaded(0)
        for i in range(n + 4):
            if i < n:
                a, P_ = flat[i]
                gi = group_of[a]
                ensure_loaded(gi + 1)
                st = dict(a=a, P_=P_, wE=P_ + 2, gi=gi,
                          la=a - cur[gi]["ga"],
                          d2s_on_act=(i in (11, 22)),
                          tail=(i >= n - 3))
                sts.append(st)
                front_a(st)
                flush_hs()
            if 0 <= i - 3 < n:
                back_dve(sts[i - 3])
            if 0 <= i - 1 < n:
                front_b(sts[i - 1])
            if 0 <= i - 3 < n:
                back_rest(sts[i - 3])
            if 0 <= i - 4 < n:
                out_relu(sts[i - 4])

    nc.compile()
    return nc


def kernel(x, w_b1_dw, w_b1_pw, w_b2_1x1, w_b2_dw, w_b2_pw, w_fusion):
    global LAST_RESULT, _nc_cache

    x = np.asarray(x, dtype=np.float32)
    h = np.ascontiguousarray(x.T)
    mask = _mask_cn()

    # host-side shard prep: [C, N] layouts, zero-padded halos
    x8_pad = np.zeros((C, N + 8), dtype=NP8)
    x8_pad[:, 2:N + 2] = h.astype(NP8)
    x16_pad = h.astype(np.float16)
    mk_pad = np.zeros((C, N + 2), dtype=NP8)
    mk_pad[:, 1:N + 1] = mask.astype(NP8)

    def taps(wdw):  # [C,1,3,3] -> per-channel taps along N
        return np.asarray(wdw)[:, 0, :, 1]  # [C, 3]

    t1 = taps(w_b1_dw)
    t2 = taps(w_b2_dw)

    def diag8(v):
        return np.diag(v.astype(np.float32)).astype(NP8)

    def lhsT8(w):  # [O, I] -> [I, O] fp8
        return np.ascontiguousarray(np.asarray(w, dtype=np.float32).T).astype(NP8)

    zero = np.zeros((C, C), dtype=NP8)
    pairs = [
        (diag8(t1[:, 0]), diag8(t1[:, 2])),
        (diag8(t1[:, 1]), zero),
        (lhsT8(np.asarray(w_b1_pw)[:, :, 0, 0]), zero),
        (lhsT8(np.asarray(w_b2_1x1)[:, :, 0, 0]), zero),
        (diag8(t2[:, 0]), diag8(t2[:, 2])),
        (diag8(t2[:, 1]), zero),
        (lhsT8(np.asarray(w_b2_pw)[:, :, 0, 0]), zero),
        (lhsT8(np.asarray(w_fusion)[:, C:, 0, 0]), zero),
    ]
    w8_host = np.empty((C, 8 * 2 * C), dtype=NP8)
    for k, (p0, p1) in enumerate(pairs):
        w8_host[:, (2 * k) * C:(2 * k + 1) * C] = p0
        w8_host[:, (2 * k + 1) * C:(2 * k + 2) * C] = p1
    wf_host = np.ascontiguousarray(
        np.asarray(w_fusion)[:, :C, 0, 0].astype(np.float32).T
    ).astype(np.float16)

    in_maps = []
    for i in range(NCORES):
        s = i * NSH
        in_maps.append({
            "x8": np.ascontiguousarray(x8_pad[:, s:s + NSH + 8]),
            "x16": np.ascontiguousarray(x16_pad[:, s:s + NSH]),
            "mk": np.ascontiguousarray(mk_pad[:, s:s + NSH + 2]),
            "w8": w8_host,
            "wf16": wf_host,
        })

    if _nc_cache is None:
        _nc_cache = _build_nc()

    res = run_bass_kernel_spmd(
        _nc_cache, in_maps, core_ids=list(range(NCORES)), trace=TRACE
    )
    LAST_RESULT = res

    out = np.empty((C, N), dtype=np.float32)
    for i in range(NCORES):
        out[:, i * NSH:(i + 1) * NSH] = res.results[i]["y"].astype(np.float32)
    return np.ascontiguousarray(out.T)


# revision 30
# speedup vs baseline: 1.3395x; 1.0264x over previous
"""Trainium2 Bass kernel for nn_KB_Mapping_19361712570541 (dense_cnn).

Math (W=1 image dim folded away; h = x.T in [C, N] channels-on-partition):
  dw3(h, w)[c,n] = w0[c]h[c,n-1] + w1[c]h[c,n] + w2[c]h[c,n+1]   (zero pad)
  b1  = relu(W1pw @ relu(dw3(h, wd1)))
  b2  = (relu(W21x1 @ h) + b1) * mask
  b2  = relu(W2pw @ relu(dw3(b2, wd2)))
  out = relu(Wf[:, :C] @ h + Wf[:, C:] @ b2)          -> out.T is [N, C]

Sharding: data-parallel along N across 8 cores; each core's input slab
carries halos of x/mask so no cross-core communication is needed.

Implementation notes (cost-model driven):
- All matmuls except the fusion's Wfh@h run as fp8e4 DoubleRow pairs
  (0.5 cycles/row, two K=128 planes per instruction). Depthwise taps
  pair {t0,t2} (plane stride +2 — odd plane strides are rejected by
  codegen) with {t1, zero-plane} as the second DR; single GEMMs ride a
  zero second weight plane at half cost. The accuracy-critical fusion
  term Wfh@h runs in fp16 (x shipped both as fp8 and fp16); measured
  end-to-end rel err ~6e-4.
- Six PSUM->SBUF evacuations per 510-col tile are the floor: DVE takes
  {relu d1, relu+add (STT), relu d2}, ACT {relu b1, relu b2, relu out};
  the mask multiply runs on Pool. One PSUM bank per tensor (d1 and the
  fusion accumulator double-buffered; 8 banks total).
- The emission is software-pipelined with skew: iteration i emits
  front_a(i) (dw1/d1s/b1p/b1r), back_dve(i-3) (dw2/d2s), front_b(i-1)
  (b2a/STT/mask), back_rest(i-3) (b2p/b2r/fusion), out_relu(i-4) — so
  per iteration the in-order queues see DVE:[d1s(i), d2s(i-3),
  STT(i-1)] and ACT:[b1r(i), b2r(i-3), o(i-4)] with every dependency
  (including the ~1.3us Pool mask latency) already satisfied. Two
  mid-stream tiles run d2s on ACT for balance; the last three tiles run
  b2r on DVE because ACT serializes the drain.
- DMAs are batched into graduated groups (sizes hill-climbed; each
  dma_start costs ~625ns of serial HWDGE time); the fp16 fusion slab
  rides one iteration later than the critical fp8 slab, and a narrow
  leading tile plus narrow trailing tiles shorten pipeline fill/drain.
  Work-tile ring depths (b2b 3, b1r 4, out 3) are tuned: extra slack
  there removes WAR hiccups, but deeper rings on d1s/d2s/b2r hurt.
- Zero-plane DRs read up to 2 columns past the producer's content; the
  fp8 work tiles are 516 wide and fully memset once per buffer (the
  interpreter hard-faults on uninitialized reads).
"""

import numpy as np
from contextlib import ExitStack

import ml_dtypes

import concourse.bass as bass
import concourse.bacc as bacc
import concourse.tile as tile
import concourse.mybir as mybir
from concourse.ap import AP
from concourse.bass_utils import run_bass_kernel_spmd

C = 128
N = 131072
NCORES = 8
NSH = N // NCORES          # 16384 output columns per core
T = 510                    # steady-state tile width (wE = 512 = one PSUM bank)
MASK_SEED = 42
MASK_P = 0.5

F32 = mybir.dt.float32
F16 = mybir.dt.float16
F8 = mybir.dt.float8e4
NP8 = ml_dtypes.float8_e4m3
DR = mybir.MatmulPerfMode.DoubleRow
Relu = mybir.ActivationFunctionType.Relu

# DR weight-pair indices in w8 (each pair is [C, 2, C] -> 256 cols)
P_D1A, P_D1B, P_W1, P_W21, P_D2A, P_D2B, P_W2, P_WFB = range(8)

LAST_RESULT = None         # BassKernelResults of the most recent run (for test.py)
TRACE = False

_mask_cache = None
_nc_cache = None


def _mask_cn() -> np.ndarray:
    """The reference's fixed Bernoulli mask in [C, N] layout, float32."""
    global _mask_cache
    if _mask_cache is None:
        import jax
        cpu = jax.devices("cpu")[0]
        with jax.default_device(cpu):
            m = jax.random.bernoulli(
                jax.random.key(MASK_SEED), 1.0 - MASK_P, (1, C, N, 1)
            )
            _mask_cache = np.asarray(m)[0, :, :, 0].astype(np.float32)
    return _mask_cache


def _tiles():
    """(a, width) list covering [0, NSH); narrow leader fills the pipe and
    narrow trailers drain it."""
    widths = [256] + [T] * 30 + [276] * 3
    assert sum(widths) == NSH
    out, a = [], 0
    for w in widths:
        out.append((a, w))
        a += w
    return out


def _groups(tiles):
    """Graduated DMA groups as slices of the tile list."""
    sizes = [2, 2, 4, 8, 8, 9]
    gs, i = [], 0
    for s in sizes:
        if i >= len(tiles):
            break
        gs.append(tiles[i:i + s])
        i += s
    if i < len(tiles):
        gs.append(tiles[i:])
    return gs


def _dr_rhs(t, col, n, delta=2):
    """[C, 2, n] moving AP over tile t: plane0 at col, plane1 at col+delta."""
    base = t[:, col:col + n]
    return AP(base.tensor, base.offset,
              [list(base.ap[0]), [delta, 2], [1, n]])


def _build_nc():
    nc = bacc.Bacc("TRN2", target_bir_lowering=False)

    x8 = nc.dram_tensor("x8", [C, NSH + 8], F8, kind="ExternalInput")
    x16 = nc.dram_tensor("x16", [C, NSH], F16, kind="ExternalInput")
    mk = nc.dram_tensor("mk", [C, NSH + 2], F8, kind="ExternalInput")
    w8 = nc.dram_tensor("w8", [C, 8 * 2 * C], F8, kind="ExternalInput")
    wf16 = nc.dram_tensor("wf16", [C, C], F16, kind="ExternalInput")
    y = nc.dram_tensor("y", [C, NSH], F16, kind="ExternalOutput")

    tiles = _tiles()
    groups = _groups(tiles)
    gw_max = max(sum(w for _, w in g) for g in groups)
    group_of = {}
    for gi, g in enumerate(groups):
        for t_ in g:
            group_of[t_[0]] = gi

    with ExitStack() as ctx:
        tc = ctx.enter_context(tile.TileContext(nc))
        wpool = ctx.enter_context(tc.tile_pool(name="weights", bufs=1))
        slab = ctx.enter_context(tc.tile_pool(name="slab", bufs=3))
        opool = ctx.enter_context(tc.tile_pool(name="out", bufs=3))
        work = ctx.enter_context(tc.tile_pool(name="work", bufs=3))
        ps = ctx.enter_context(tc.tile_pool(name="ps", bufs=1, space="PSUM"))

        w8_sb = wpool.tile([C, 8 * 2 * C], F8)
        wf_sb = wpool.tile([C, C], F16)

        def wpair(k):
            return w8_sb[:, k * 2 * C:(k + 1) * 2 * C].rearrange(
                "p (two m) -> p two m", two=2)

        # One-time zero of every buffer whose tail columns are read by
        # zero-plane DRs (delta-2 planes reach 2 cols past the written
        # content; the interpreter hard-faults on uninitialized reads).
        ZBUFS = {"d1s": 2, "b2m": 3, "d2s": 2, "b2r": 2}
        for ztag, zb in ZBUFS.items():
            for _ in range(zb):
                zt = work.tile([C, 516], F8, tag=ztag, bufs=zb,
                               name=f"z_{ztag}")
                nc.gpsimd.memset(zt[:, :], 0.0)

        # per-group slab state
        cur = {}

        pending_hs = []

        def load_group(gi):
            g = groups[gi]
            ga = g[0][0]
            gw = sum(w for _, w in g)
            xs = slab.tile([C, gw_max + 6], F8, tag="xs")
            nc.sync.dma_start(out=xs[:, :gw + 6], in_=x8[:, ga:ga + gw + 6])
            if gi == 0:
                # interleave the weight loads so dw1's pairs (first 768
                # cols) land right after the first x slab: the leading
                # tile's matmuls start ~2 HWDGE slots in.
                nc.sync.dma_start(out=w8_sb[:, :512], in_=w8[:, :512])
            ms = slab.tile([C, gw_max + 2], F8, tag="ms")
            nc.sync.dma_start(out=ms[:, :gw + 2], in_=mk[:, ga:ga + gw + 2])
            if gi == 0:
                nc.sync.dma_start(out=w8_sb[:, 512:], in_=w8[:, 512:])
            # the fp16 slab is only read by the fusion (lag 3): defer its
            # DMA so the next group's critical x slab gets the HWDGE first
            hs = slab.tile([C, gw_max], F16, tag="hs")
            o_c = opool.tile([C, gw_max], F16, tag="oc")
            cur[gi] = dict(ga=ga, gw=gw, xs=xs, hs=hs, ms=ms, o_c=o_c,
                           flushed=0, done=0)
            pending_hs.append(gi)

        def flush_hs():
            while pending_hs:
                gi = pending_hs.pop(0)
                g = cur[gi]
                nc.sync.dma_start(out=g["hs"][:, :g["gw"]],
                                  in_=x16[:, g["ga"]:g["ga"] + g["gw"]])
                if gi == 0:
                    nc.sync.dma_start(out=wf_sb[:, :], in_=wf16[:, :])

        def front_a(st):
            """dw1 -> d1s (DVE) -> b1p -> b1r (ACT)."""
            g = cur[st["gi"]]
            la, wE = st["la"], st["wE"]
            xs = g["xs"]
            d1p = ps.tile([C, 512], F32, tag="d1", bufs=2, name="d1p")
            nc.tensor.matmul(d1p[:, :wE], wpair(P_D1A),
                             _dr_rhs(xs, la, wE),
                             start=True, stop=False, perf_mode=DR)
            nc.tensor.matmul(d1p[:, :wE], wpair(P_D1B),
                             _dr_rhs(xs, la + 1, wE),
                             start=False, stop=True, perf_mode=DR)
            d1s = work.tile([C, 516], F8, tag="d1s", bufs=2)
            nc.vector.tensor_scalar_max(d1s[:, :wE], d1p[:, :wE], 0.0)
            b1p = ps.tile([C, 512], F32, tag="b1", name="b1p")
            nc.tensor.matmul(b1p[:, :wE], wpair(P_W1), _dr_rhs(d1s, 0, wE),
                             start=True, stop=True, perf_mode=DR)
            b1r = work.tile([C, 512], F16, tag="b1r", bufs=3)
            nc.scalar.activation(b1r[:, :wE], b1p[:, :wE], Relu)
            st.update(b1r=b1r)

        def back_dve(st):
            """dw2 -> d2s (DVE; ACT on a couple of tiles for balance)."""
            P_ = st["P_"]
            d2p = ps.tile([C, 512], F32, tag="d2", name="d2p")
            b2m = st["b2m"]
            nc.tensor.matmul(d2p[:, :P_], wpair(P_D2A), _dr_rhs(b2m, 0, P_),
                             start=True, stop=False, perf_mode=DR)
            nc.tensor.matmul(d2p[:, :P_], wpair(P_D2B), _dr_rhs(b2m, 1, P_),
                             start=False, stop=True, perf_mode=DR)
            d2s = work.tile([C, 516], F8, tag="d2s", bufs=2)
            if st.get("d2s_on_act"):
                nc.scalar.activation(d2s[:, :P_], d2p[:, :P_], Relu)
            else:
                nc.vector.tensor_scalar_max(d2s[:, :P_], d2p[:, :P_], 0.0)
            st.update(d2s=d2s)

        def front_b(st):
            """b2a -> STT (DVE) -> mask (Pool)."""
            g = cur[st["gi"]]
            la, wE = st["la"], st["wE"]
            b2ap = ps.tile([C, 512], F32, tag="b2a", name="b2ap")
            nc.tensor.matmul(b2ap[:, :wE], wpair(P_W21),
                             _dr_rhs(g["xs"], la + 1, wE),
                             start=True, stop=True, perf_mode=DR)
            b2b = work.tile([C, 512], F16, tag="b2b", bufs=3)
            nc.vector.scalar_tensor_tensor(
                b2b[:, :wE], b2ap[:, :wE], 0.0, st["b1r"][:, :wE],
                mybir.AluOpType.max, mybir.AluOpType.add)
            b2m = work.tile([C, 516], F8, tag="b2m", bufs=3)
            nc.gpsimd.tensor_mul(b2m[:, :wE], b2b[:, :wE],
                                 g["ms"][:, la:la + wE])
            st.update(b2m=b2m)

        def back_rest(st):
            """b2p -> b2r (ACT) -> fusion matmuls."""
            g = cur[st["gi"]]
            la, P_ = st["la"], st["P_"]
            b2p = ps.tile([C, 512], F32, tag="b2", name="b2p")
            nc.tensor.matmul(b2p[:, :P_], wpair(P_W2),
                             _dr_rhs(st["d2s"], 0, P_),
                             start=True, stop=True, perf_mode=DR)
            b2r = work.tile([C, 516], F8, tag="b2r", bufs=2)
            if st.get("tail"):
                nc.vector.tensor_scalar_max(b2r[:, :P_], b2p[:, :P_], 0.0)
            else:
                nc.scalar.activation(b2r[:, :P_], b2p[:, :P_], Relu)
            fp = ps.tile([C, 512], F32, tag="f", bufs=2, name="fp")
            nc.tensor.matmul(fp[:, :P_], wf_sb[:, :], g["hs"][:, la:la + P_],
                             start=True, stop=False)
            nc.tensor.matmul(fp[:, :P_], wpair(P_WFB), _dr_rhs(b2r, 0, P_),
                             start=False, stop=True, perf_mode=DR)
            st.update(fp=fp)

        def out_relu(st):
            """final relu (ACT) + output flush bookkeeping."""
            g = cur[st["gi"]]
            la, P_ = st["la"], st["P_"]
            if st.get("last"):
                nc.vector.tensor_scalar_max(g["o_c"][:, la:la + P_],
                                            st["fp"][:, :P_], 0.0)
            else:
                nc.scalar.activation(g["o_c"][:, la:la + P_], st["fp"][:, :P_],
                                     Relu)
            g["done"] += 1
            ntiles = len(groups[st["gi"]])
            # flush every 2 finished tiles
            if g["done"] % 2 == 0 or g["done"] == ntiles:
                lo, hi = g["flushed"], la + P_
                nc.sync.dma_start(out=y[:, g["ga"] + lo:g["ga"] + hi],
                                  in_=g["o_c"][:, lo:hi])
                g["flushed"] = hi

        # software-pipelined emission with skew: per iteration i the engine
        # queues get  DVE:[d1s(i), d2s(i-3), STT(i-1)]
        #             ACT:[b1r(i), b2r(i-3), o(i-4)]
        # so no engine waits on a cross-engine round trip: the Pool mask op
        # of tile i-3 has had two full iterations to finish before dw2/d2s.
        flat = [t_ for g in groups for t_ in g]
        n = len(flat)
        sts = []
        loaded = 0

        def ensure_loaded(upto):
            nonlocal loaded
            while loaded <= min(upto, len(groups) - 1):
                load_group(loaded)
                loaded += 1

        ensure_loaded(0)
        for i in range(n + 4):
            if i < n:
                a, P_ = flat[i]
                gi = group_of[a]
                ensure_loaded(gi + 1)
                st = dict(a=a, P_=P_, wE=P_ + 2, gi=gi,
                          la=a - cur[gi]["ga"],
                          d2s_on_act=(i in (11, 22)),
                          tail=(i >= n - 3), last=(i == n - 1))
                sts.append(st)
                front_a(st)
                flush_hs()
            if 0 <= i - 3 < n:
                back_dve(sts[i - 3])
            if 0 <= i - 1 < n:
                front_b(sts[i - 1])
            if 0 <= i - 3 < n:
                back_rest(sts[i - 3])
            if 0 <= i - 4 < n:
                out_relu(sts[i - 4])

    nc.compile()
    return nc


def kernel(x, w_b1_dw, w_b1_pw, w_b2_1x1, w_b2_dw, w_b2_pw, w_fusion):
    global LAST_RESULT, _nc_cache

    x = np.asarray(x, dtype=np.float32)
    h = np.ascontiguousarray(x.T)
    mask = _mask_cn()

    # host-side shard prep: [C, N] layouts, zero-padded halos
    x8_pad = np.zeros((C, N + 8), dtype=NP8)
    x8_pad[:, 2:N + 2] = h.astype(NP8)
    x16_pad = h.astype(np.float16)
    mk_pad = np.zeros((C, N + 2), dtype=NP8)
    mk_pad[:, 1:N + 1] = mask.astype(NP8)

    def taps(wdw):  # [C,1,3,3] -> per-channel taps along N
        return np.asarray(wdw)[:, 0, :, 1]  # [C, 3]

    t1 = taps(w_b1_dw)
    t2 = taps(w_b2_dw)

    def diag8(v):
        return np.diag(v.astype(np.float32)).astype(NP8)

    def lhsT8(w):  # [O, I] -> [I, O] fp8
        return np.ascontiguousarray(np.asarray(w, dtype=np.float32).T).astype(NP8)

    zero = np.zeros((C, C), dtype=NP8)
    pairs = [
        (diag8(t1[:, 0]), diag8(t1[:, 2])),
        (diag8(t1[:, 1]), zero),
        (lhsT8(np.asarray(w_b1_pw)[:, :, 0, 0]), zero),
        (lhsT8(np.asarray(w_b2_1x1)[:, :, 0, 0]), zero),
        (diag8(t2[:, 0]), diag8(t2[:, 2])),
        (diag8(t2[:, 1]), zero),
        (lhsT8(np.asarray(w_b2_pw)[:, :, 0, 0]), zero),
        (lhsT8(np.asarray(w_fusion)[:, C:, 0, 0]), zero),
    ]
    w8_host = np.empty((C, 8 * 2 * C), dtype=NP8)
    for k, (p0, p1) in enumerate(pairs):
        w8_host[:, (2 * k) * C:(2 * k + 1) * C] = p0
        w8_host[:, (2 * k + 1) * C:(2 * k + 2) * C] = p1
    wf_host = np.ascontiguousarray(
        np.asarray(w_fusion)[:, :C, 0, 0].astype(np.float32).T
    ).astype(np.float16)

    in_maps = []
    for i in range(NCORES):
        s = i * NSH
        in_maps.append({
            "x8": np.ascontiguousarray(x8_pad[:, s:s + NSH + 8]),
            "x16": np.ascontiguousarray(x16_pad[:, s:s + NSH]),
            "mk": np.ascontiguousarray(mk_pad[:, s:s + NSH + 2]),
            "w8": w8_host,
            "wf16": wf_host,
        })

    if _nc_cache is None:
        _nc_cache = _build_nc()

    res = run_bass_kernel_spmd(
        _nc_cache, in_maps, core_ids=list(range(NCORES)), trace=TRACE
    )
    LAST_RESULT = res

    out = np.empty((C, N), dtype=np.float32)
    for i in range(NCORES):
        out[:, i * NSH:(i + 1) * NSH] = res.results[i]["y"].astype(np.float32)
    return np.ascontiguousarray(out.T)
